# revision 43
# baseline (speedup 1.0000x reference)
"""MoE layer (GShard top-2 routing + per-expert FFN) on 8 Trainium2 NeuronCores.

Strategy (expert parallelism, ReduceScatter return path; cost-model time
~602us vs ~951us for the AllGather+combine baseline):
  - Router matmul (fp32, exact) is token-sharded: each core computes logits
    for its 1024-token shard, then a small bf16 AllGather shares per-token
    routing scalars (idx1, idx2, g1, g2) with all cores.
  - Every core replicates the (cheap) global slot-assignment math, batched
    over the two routing choices: per-(choice, expert) inclusive scans along
    the free dim + a triangular-matmul partition prefix give each token its
    capacity slot exactly as the reference's cumsum does.
  - Each core owns ONE expert. The slot->token map AND the per-slot gates
    (bf16, keep folded) are built with local_scatter (per-partition scatter
    by slot), merged across partitions with a ones-matmul, and read out
    column-major via a diagonal extraction. tokci cols 0-3 are extracted
    first so block-0 dispatch starts before the rest is done.
  - Dispatch: 16 indirect row gathers from x (bf16) + XBAR DMA transposes
    (dma_start_transpose) build the [d, slot] layout entirely off the
    in-order PE queue. FFN in bf16 with fp32 accumulation:
    hT = gelu(w_gate^T @ dispT), eo = g_slot * (hT^T @ w_down) with the gate
    multiply folded into the PSUM->SBUF copy.
  - Return: each block's gated eo rows are indirect-scattered to rs_in[token]
    (empty/dropped slots fall on row T and are dropped by bounds_check); a
    single ReduceScatter(add) over [T, D] bf16 sums the two expert
    contributions per token and leaves shard m's rows on core m (collective
    cost scales with OUTPUT size: 2MB -> ~67us vs the 32MB AllGather's 320us).
  - Scheduling (Tile orders by dependency, not program order, and its global
    clock is enforced at runtime): block-1 gather offsets and the gate
    scatters carry artificial RAW edges on block-0's dispT; the bulk wdn /
    rs_in-zero-fill DMAs are gated on block-1's last gather so they cannot
    crowd the serialized DMA queue ahead of critical dispatch; the gate-merge
    matmuls are created between mm1(b0) and mm2(b0) so they sit behind mm1 in
    the PE queue; rs_in is zeroed with doubling DRAM->DRAM copies (8 DMA
    instructions) to keep semaphore-lane reuse pressure low.
"""

import sys

if "/opt/trn_rl_repo" not in sys.path:
    sys.path.insert(0, "/opt/trn_rl_repo")

import numpy as np
import ml_dtypes

import concourse.bacc as bacc
import concourse.mybir as mybir
import concourse.tile as tile
from concourse import bass
from concourse.bass_utils import run_bass_kernel_spmd

BF16 = mybir.dt.bfloat16
F32 = mybir.dt.float32
I16 = mybir.dt.int16
I32 = mybir.dt.int32
AF = mybir.ActivationFunctionType
OP = mybir.AluOpType

B, S, D, E, F = 4, 2048, 1024, 8, 4096
T = B * S            # 8192 tokens
C = 2 * T // E       # 2048 capacity
NC = 8               # cores
SH = T // NC         # 1024 tokens per shard
CBLK = 512           # FFN slot-block
NCB = C // CBLK      # 4 blocks

LAST_RESULT = None   # BassKernelResults of the most recent run (for profiling)


def _build_program():
    nc = bacc.Bacc("TRN2", target_bir_lowering=False, debug=False, num_devices=NC)

    # ---- per-core external inputs ----
    xT_sh = nc.dram_tensor("xT_sh", [D, SH], F32, kind="ExternalInput").ap()
    xb = nc.dram_tensor("xb", [T + 1, D], BF16, kind="ExternalInput").ap()
    wg_d = nc.dram_tensor("wg", [D, E], F32, kind="ExternalInput").ap()
    wgt_d = nc.dram_tensor("wgt", [D, F], BF16, kind="ExternalInput").ap()
    wdn_d = nc.dram_tensor("wdn", [F, D], BF16, kind="ExternalInput").ap()
    cid_d = nc.dram_tensor("cid", [128, 1], F32, kind="ExternalInput").ap()
    # host-generated constants (gpsimd iota/affine_select aren't available)
    ident_d = nc.dram_tensor("ident", [128, 128], F32, kind="ExternalInput").ap()
    slmat_d = nc.dram_tensor("slmat", [128, 128], F32, kind="ExternalInput").ap()
    tidx_d = nc.dram_tensor("tidx", [128, 64], F32, kind="ExternalInput").ap()
    eidx_d = nc.dram_tensor("eidx", [128, E], F32, kind="ExternalInput").ap()
    y_d = nc.dram_tensor("y", [SH, D], BF16, kind="ExternalOutput").ap()

    # ---- internal DRAM ----
    pay_in = nc.dram_tensor("pay_in", [4 * SH], BF16).ap()
    pay_all = nc.dram_tensor("pay_all", [NC * 4 * SH], BF16, addr_space="Shared").ap()
    rs_in = nc.dram_tensor("rs_in", [T, D], BF16).ap()
    rs_out = nc.dram_tensor("rs_out", [SH, D], BF16).ap()

    with tile.TileContext(nc) as tc:
        with (
            tc.tile_pool(name="persist", bufs=1) as pp,
            tc.tile_pool(name="psum_s", bufs=2, space="PSUM") as pss,
        ):
            cid = pp.tile([128, 1], F32)
            # zero-fill seed for rs_in (zeroed via a gated write + DRAM->DRAM
            # doubling copies, so only 8 DMA instructions and no early queue
            # crowding)
            zt = pp.tile([128, 1, D], BF16)

            # resident gate weight (bf16); DMAs issued after the router
            # section (chunked so small DMAs can interleave); wdn_sb lives in
            # the FFN pool (not needed until mm2) to relieve SBUF pressure
            wgt_sb = pp.tile([128, D // 128, F], BF16)

            # persistent routing products
            tokci = pp.tile([128, C // 128], I32)   # dispatch+return row (t or T)
            gsl = pp.tile([128, C // 128], F32)     # per-slot gate (keep folded)
            gdata = pp.tile([128, 128], BF16)       # per-token gates for scatter
            idxlo = pp.tile([128, 128], I16)        # scatter idx, slots < 1024
            idxhi = pp.tile([128, 128], I16)        # scatter idx, slots >= 1024
            ones_bf = pp.tile([128, 128], BF16)
            nc.vector.memset(ones_bf[:], 1.0)
            identp = pp.tile([128, 128], BF16)      # identity (diag extraction)

            # =============== ROUTER (token shard, fp32) ===============
            with tc.tile_pool(name="route", bufs=1) as pr:
                xT_sb = pr.tile([128, D // 128, SH], F32)
                nc.sync.dma_start(xT_sb[:], xT_sh.rearrange("(o q) t -> q o t", q=128))
                wg_sb = pr.tile([128, D // 128, E], F32)
                nc.sync.dma_start(wg_sb[:], wg_d.rearrange("(o q) e -> q o e", q=128))
                ident = pr.tile([128, 128], F32)
                nc.sync.dma_start(ident[:], ident_d[:])
                nc.sync.dma_start(cid[:], cid_d[:])
                zeros64 = pr.tile([128, 64], F32)
                nc.vector.memset(zeros64[:], 0.0)
                ones128 = pr.tile([128, 128], F32)
                nc.vector.memset(ones128[:], 1.0)

                lg = pr.tile([128, 8, E], F32)  # logits, token pos j = 128*tt + p
                for tt in range(8):
                    ps = pss.tile([128, E], F32, space="PSUM", tag="ps_small")
                    for kd in range(8):
                        nc.tensor.matmul(
                            ps[:],
                            lhsT=xT_sb[:, kd, 128 * tt : 128 * tt + 128],
                            rhs=wg_sb[:, kd, :],
                            start=(kd == 0),
                            stop=(kd == 7),
                        )
                    nc.vector.tensor_copy(lg[:, tt, :], ps[:])

                def emax(src, width, tag):
                    cur = src
                    w = width
                    while w > 1:
                        nxt = pr.tile([128, 8, w // 2], F32, tag=f"emax{tag}{w}")
                        nc.vector.tensor_tensor(
                            out=nxt[:], in0=cur[:, :, : w // 2], in1=cur[:, :, w // 2 :],
                            op=OP.max,
                        )
                        cur, w = nxt, w // 2
                    return cur  # [128, 8, 1]

                m1x = emax(lg, E, "m1")
                is1 = pr.tile([128, 8, E], F32)
                nc.vector.tensor_tensor(out=is1[:], in0=lg[:], in1=m1x[:].to_broadcast([128, 8, E]), op=OP.is_equal)
                l2 = pr.tile([128, 8, E], F32)
                nc.vector.scalar_tensor_tensor(
                    out=l2[:], in0=is1[:], scalar=-1e30, in1=lg[:], op0=OP.mult, op1=OP.add,
                )
                m2x = emax(l2, E, "m2")
                is2 = pr.tile([128, 8, E], F32)
                nc.vector.tensor_tensor(out=is2[:], in0=l2[:], in1=m2x[:].to_broadcast([128, 8, E]), op=OP.is_equal)

                dm = pr.tile([128, 8, 1], F32)
                nc.vector.tensor_tensor(out=dm[:], in0=m2x[:], in1=m1x[:], op=OP.subtract)
                e2 = pr.tile([128, 8, 1], F32)
                nc.scalar.activation(e2[:], dm[:], AF.Exp)
                den = pr.tile([128, 8, 1], F32)
                nc.vector.tensor_scalar_add(den[:], e2[:], 1.0)
                g1 = pr.tile([128, 8, 1], F32)
                nc.vector.reciprocal(g1[:], den[:])
                g2 = pr.tile([128, 8, 1], F32)
                nc.vector.tensor_tensor(out=g2[:], in0=e2[:], in1=g1[:], op=OP.mult)

                eidx = pr.tile([128, E], F32)
                nc.sync.dma_start(eidx[:], eidx_d[:])

                def argmax_num(mask, tag):
                    t1 = pr.tile([128, 8, E], F32, tag=f"am_t1{tag}")
                    nc.vector.tensor_tensor(
                        out=t1[:], in0=mask[:], in1=eidx[:, None, :].to_broadcast([128, 8, E]), op=OP.mult,
                    )
                    cur, w = t1, E
                    while w > 1:
                        nxt = pr.tile([128, 8, w // 2], F32, tag=f"am_s{tag}{w}")
                        nc.vector.tensor_tensor(
                            out=nxt[:], in0=cur[:, :, : w // 2], in1=cur[:, :, w // 2 :], op=OP.add,
                        )
                        cur, w = nxt, w // 2
                    return cur  # [128, 8, 1]

                i1f = argmax_num(is1, "a")
                i2f = argmax_num(is2, "b")

                pk = pr.tile([128, 4, 8], BF16)
                nc.vector.tensor_copy(pk[:, 0, :], i1f[:, :, 0])
                nc.vector.tensor_copy(pk[:, 1, :], i2f[:, :, 0])
                nc.vector.tensor_copy(pk[:, 2, :], g1[:, :, 0])
                nc.vector.tensor_copy(pk[:, 3, :], g2[:, :, 0])
                nc.sync.dma_start(pay_in.rearrange("(a p tt) -> p a tt", a=4, p=128), pk[:])

                nc.gpsimd.collective_compute(
                    "AllGather", OP.bypass,
                    replica_groups=[list(range(NC))],
                    ins=[pay_in[:].opt()], outs=[pay_all[:].opt()],
                )

                # reread all 4 arrays into global routing layout [128, 64] (t = 64p + i)
                rt = pr.tile([128, 4, 64], BF16)
                pay_view = pay_all.rearrange("(r a p16 i) -> r p16 a i", r=NC, a=4, p16=16)
                for r in range(NC):
                    nc.sync.dma_start(rt[16 * r : 16 * r + 16, :, :], pay_view[r])
                # =============== SLOT ASSIGNMENT (replicated) ===============
                # choice dim c (top-1 / top-2) batched: rt[:, 0:2] = experts,
                # rt[:, 2:4] = gates
                m12 = pr.tile([128, 2, E, 64], F32)
                sc12 = pr.tile([128, 2, E, 64], F32)
                nc.vector.tensor_tensor(
                    out=m12[:],
                    in0=rt[:, 0:2, None, :].to_broadcast([128, 2, E, 64]),
                    in1=eidx[:, None, :, None].to_broadcast([128, 2, E, 64]),
                    op=OP.is_equal,
                )
                for c in range(2):
                    for e in range(E):
                        nc.vector.tensor_tensor_scan(sc12[:, c, e, :], m12[:, c, e, :], zeros64[:], 0.0, op0=OP.add, op1=OP.add)
                tot12 = pr.tile([128, 2, E], F32)
                nc.vector.tensor_copy(tot12[:], sc12[:, :, :, 63])

                sl = pr.tile([128, 128], F32)
                nc.sync.dma_start(sl[:], slmat_d[:])

                # of[c] = exclusive-prefix-over-groups of tot[c]; choice 2 also
                # offset by the global top-1 count per expert
                of12_ps = pss.tile([128, 2, E], F32, space="PSUM", tag="ps_small")
                nc.tensor.matmul(of12_ps[:].rearrange("p c e -> p (c e)"), lhsT=sl[:], rhs=tot12[:].rearrange("p c e -> p (c e)"), start=True, stop=False)
                nc.tensor.matmul(of12_ps[:, 1, :], lhsT=ones128[:], rhs=tot12[:, 0, :], start=False, stop=True)
                of12 = pr.tile([128, 2, E], F32)
                nc.vector.tensor_scalar_add(of12[:], of12_ps[:], -1.0)

                # loc = (inclusive_scan + offset - 1) masked to own expert,
                # summed over e
                lt = pr.tile([128, 2, E, 64], F32)
                nc.vector.tensor_tensor(
                    out=lt[:], in0=sc12[:], in1=of12[:, :, :, None].to_broadcast([128, 2, E, 64]), op=OP.add,
                )
                nc.vector.tensor_tensor(out=lt[:], in0=lt[:], in1=m12[:], op=OP.mult)
                cur, w = lt, E
                while w > 1:
                    nxt = pr.tile([128, 2, w // 2, 64], F32, tag=f"loc_s{w}")
                    nc.vector.tensor_tensor(out=nxt[:], in0=cur[:, :, : w // 2, :], in1=cur[:, :, w // 2 :, :], op=OP.add)
                    cur, w = nxt, w // 2
                l12s = cur[:, :, 0, :]  # [128, 2, 64]

                kp12 = pr.tile([128, 2, 64], F32)
                nc.vector.tensor_scalar(out=kp12[:], in0=l12s, scalar1=float(C), scalar2=None, op0=OP.is_lt)
                gk12 = pr.tile([128, 2, 64], F32)
                nc.vector.tensor_tensor(out=gk12[:], in0=rt[:, 2:4, :], in1=kp12[:], op=OP.mult)
                g1k, g2k = gk12[:, 0, :], gk12[:, 1, :]

                # ====== SLOT -> GID MAP (local_scatter + merge + diagonal) ======
                tif = pr.tile([128, 64], F32)
                nc.sync.dma_start(tif[:], tidx_d[:])
                tp1 = pr.tile([128, 64], F32)
                nc.vector.tensor_scalar_add(tp1[:], tif[:], 1.0)            # t + 1

                # sel = (expert == cid) && kept; slot+1 where selected else 0
                isc = pr.tile([128, 2, 64], F32)
                nc.vector.tensor_tensor(out=isc[:], in0=rt[:, 0:2, :], in1=cid[:, 0:1, None].to_broadcast([128, 2, 64]), op=OP.is_equal)
                sel = pr.tile([128, 2, 64], F32)
                nc.vector.tensor_tensor(out=sel[:], in0=isc[:], in1=kp12[:], op=OP.mult)
                sp1 = pr.tile([128, 2, 64], F32)
                nc.vector.tensor_scalar_add(sp1[:], l12s, 1.0)
                nc.vector.tensor_tensor(out=sp1[:], in0=sp1[:], in1=sel[:], op=OP.mult)
                # lo half: slot in [0, 1024): idx = slot, else -1
                mlo = pr.tile([128, 2, 64], F32)
                nc.vector.tensor_scalar(out=mlo[:], in0=sp1[:], scalar1=1024.0, scalar2=None, op0=OP.is_le)
                nc.vector.tensor_tensor(out=mlo[:], in0=mlo[:], in1=sel[:], op=OP.mult)
                ilo = pr.tile([128, 2, 64], F32)
                nc.vector.tensor_tensor(out=ilo[:], in0=mlo[:], in1=sp1[:], op=OP.mult)
                nc.vector.tensor_scalar_add(ilo[:], ilo[:], -1.0)
                # hi half: slot in [1024, 2048): idx = slot - 1024, else -1
                mhi = pr.tile([128, 2, 64], F32)
                nc.vector.tensor_scalar(out=mhi[:], in0=sp1[:], scalar1=1024.0, scalar2=None, op0=OP.is_gt)
                ihi = pr.tile([128, 2, 64], F32)
                nc.vector.tensor_scalar_add(ihi[:], sp1[:], -1024.0)
                nc.vector.tensor_tensor(out=ihi[:], in0=ihi[:], in1=mhi[:], op=OP.mult)
                nc.vector.tensor_scalar_add(ihi[:], ihi[:], -1.0)

                data128 = pr.tile([128, 128], I16)
                nc.vector.tensor_copy(data128[:, :64], tp1[:])
                nc.vector.tensor_copy(data128[:, 64:], tp1[:])
                nc.vector.tensor_copy(gdata[:].rearrange("p (c i) -> p c i", c=2), gk12[:])
                nc.vector.tensor_copy(idxlo[:].rearrange("p (c i) -> p c i", c=2), ilo[:])
                nc.vector.tensor_copy(idxhi[:].rearrange("p (c i) -> p c i", c=2), ihi[:])

                nc.vector.tensor_copy(identp[:], ident[:])
                dst_lo = pr.tile([128, 1024], I16)
                nc.gpsimd.local_scatter(dst_lo[:], data128[:], idxlo[:], channels=128, num_elems=1024, num_idxs=128)
                dst_hi = pr.tile([128, 1024], I16)
                nc.gpsimd.local_scatter(dst_hi[:], data128[:], idxhi[:], channels=128, num_elems=1024, num_idxs=128)

                merged = pr.tile([128, 4, 512], F32)  # gid+1 replicated on all partitions
                for half, dst in ((0, dst_lo), (1, dst_hi)):
                    dstf = pr.tile([128, 1024], F32, tag="dstf")
                    nc.vector.tensor_copy(dstf[:], dst[:])
                    for ch in range(2):
                        mg_ps = pss.tile([128, 512], F32, space="PSUM", tag="ps_small")
                        nc.tensor.matmul(mg_ps[:], lhsT=ones128[:], rhs=dstf[:, 512 * ch : 512 * (ch + 1)], start=True, stop=True)
                        nc.vector.tensor_copy(merged[:, 2 * half + ch, :], mg_ps[:])

                # diagonal extraction: tokraw[p, k] = merged-flat[128k + p];
                # cols 0-3 (block 0) first - they only need merge mm #1, so
                # block-0 dispatch can start while the rest extracts
                tokraw = pr.tile([128, C // 128], F32)
                scratch = pr.tile([128, 128], F32, tag="diag_scr")
                mview = merged[:].rearrange("p a b -> p (a b)")

                def diag_sanitize(k0, k1):
                    for k in range(k0, k1):
                        nc.vector.scalar_tensor_tensor(
                            out=scratch[:], in0=mview[:, 128 * k : 128 * (k + 1)], scalar=0.0,
                            in1=ident[:], op0=OP.add, op1=OP.mult,
                            accum_out=tokraw[:, k : k + 1],
                        )
                    # sanitize: 0 -> T+1 (empty slot -> trash row); v -> v-1
                    sl_ = slice(k0, k1)
                    iszero = pr.tile([128, k1 - k0], F32, tag=f"isz{k0}")
                    nc.vector.tensor_scalar(out=iszero[:], in0=tokraw[:, sl_], scalar1=0.0, scalar2=None, op0=OP.is_equal)
                    nc.vector.scalar_tensor_tensor(
                        out=tokraw[:, sl_], in0=iszero[:], scalar=float(T + 1), in1=tokraw[:, sl_], op0=OP.mult, op1=OP.add,
                    )
                    nc.vector.tensor_scalar_add(tokraw[:, sl_], tokraw[:, sl_], -1.0)
                    nc.vector.tensor_copy(tokci[:, sl_], tokraw[:, sl_])

                diag_sanitize(0, 4)
                diag_sanitize(4, C // 128)

            # weight loads, chunked 1MB so small DMAs interleave
            for c in range(8):
                nc.sync.dma_start(
                    wgt_sb[:, :, 512 * c : 512 * (c + 1)],
                    wgt_d[:, 512 * c : 512 * (c + 1)].rearrange("(o q) f -> q o f", q=128),
                )
            # =============== EXPERT FFN (bf16) ===============
            with (
                tc.tile_pool(name="ffn", bufs=1) as pf,
                tc.tile_pool(name="ffn_db", bufs=2) as pfd,
                tc.tile_pool(name="ffn_drow", bufs=4) as pfg,
                tc.tile_pool(name="psum_mm", bufs=2, space="PSUM") as psm,
            ):
                wdn_sb = pf.tile([128, F // 128, D], BF16)
                # 128-row strided view of rs_in for the scatters: cost-model
                # sized to what is actually written, yet overlapping every
                # zero-fill chunk so Tile orders zeros -> scatters -> RS.
                scat_view = rs_in.rearrange("(a b) d -> b a d", b=64)[0]
                eo_tiles = {}
                dispT_tiles = {}

                def dispatch(cb, off=None):
                    # gather 4 x 128 slot rows; XBAR DMA transpose into dispT
                    # (keeps dispatch off the in-order PE queue entirely):
                    # dispT[p, j, c] = drow[c, 128j + p]
                    if off is None:
                        off = (tokci, (CBLK // 128) * cb)
                    dispT = pfd.tile([128, D // 128, CBLK], BF16, tag="dispT")
                    dispT_tiles[cb] = dispT
                    for kt in range(CBLK // 128):
                        otile, obase = off
                        drow = pfg.tile([128, D], BF16, tag="drow")
                        nc.gpsimd.indirect_dma_start(
                            out=drow[:], out_offset=None, in_=xb[:],
                            in_offset=bass.IndirectOffsetOnAxis(
                                ap=otile[:, obase + kt : obase + kt + 1], axis=0),
                        )
                        nc.sync.dma_start_transpose(
                            dispT[:, :, 128 * kt : 128 * (kt + 1)], drow[:]
                        )
                    return drow

                dispatch(0)
                # block-1 offsets gated on block-0's dispT so the scheduler
                # cannot hoist block-1's gathers between block-0's transposes
                # (its clock order is enforced at runtime by sync points)
                tokb1z = pf.tile([128, 4], F32)
                nc.vector.tensor_scalar(
                    out=tokb1z[:], in0=dispT_tiles[0][:, 0, 0:4],
                    scalar1=0.0, scalar2=None, op0=OP.mult,
                )
                tokci_b1 = pf.tile([128, 4], I32)
                nc.vector.tensor_tensor(
                    out=tokci_b1[:], in0=tokci[:, 4:8], in1=tokb1z[:], op=OP.add,
                )
                dispT0_ref = dispT_tiles[0]
                drow_gate = dispatch(1, off=(tokci_b1, 0))
                # per-slot gates via the same scatter/merge/diag path (bf16
                # data; empty slots read 0 and land on the trash row anyway).
                # gdata is re-touched from dispT0 so the gate scatters cannot
                # be scheduled ahead of block-0 dispatch on the Pool engine.
                gz = pf.tile([128, 1], BF16)
                nc.vector.tensor_scalar(
                    out=gz[:], in0=dispT0_ref[:, 0, 0:1], scalar1=0.0, scalar2=None, op0=OP.mult,
                )
                nc.vector.tensor_tensor(out=gdata[:, 0:1], in0=gdata[:, 0:1], in1=gz[:], op=OP.add)
                gdst_lo = pf.tile([128, 1024], BF16)
                nc.gpsimd.local_scatter(gdst_lo[:], gdata[:], idxlo[:], channels=128, num_elems=1024, num_idxs=128)
                gdst_hi = pf.tile([128, 1024], BF16)
                nc.gpsimd.local_scatter(gdst_hi[:], gdata[:], idxhi[:], channels=128, num_elems=1024, num_idxs=128)
                # bulk loads gated behind block-1's last gather via a REAL
                # data dep (gate cells computed from drow_gate): Tile
                # schedules by dependency, not program order, so only a true
                # RAW edge keeps these DMAs out of the DMA queue until the
                # critical-path dispatch is done.
                gate_b = drow_gate[:, 0:1].to_broadcast([128, 1, D])
                for c in range(8):
                    nc.vector.tensor_scalar(
                        out=wdn_sb[:, 4 * c : 4 * c + 1, 0:1],
                        in0=drow_gate[:, 0:1], scalar1=0.0, scalar2=None, op0=OP.mult,
                    )
                    nc.sync.dma_start(
                        wdn_sb[:, 4 * c : 4 * (c + 1), :],
                        wdn_d[512 * c : 512 * (c + 1), :].rearrange("(o q) d -> q o d", q=128),
                    )
                nc.vector.tensor_scalar(
                    out=zt[:], in0=gate_b, scalar1=0.0, scalar2=None, op0=OP.mult,
                )
                nc.sync.dma_start(
                    rs_in[0:128, :].rearrange("(q p) d -> p q d", p=128), zt[:]
                )
                r = 128
                while r < T:
                    nc.sync.dma_start(rs_in[r : 2 * r, :], rs_in[0:r, :])
                    r *= 2

                for cb in range(NCB):
                    if cb >= 2:
                        dispatch(cb)
                    dispT = dispT_tiles.pop(cb)
                    if cb > 0:
                        # return previous block's gated rows to rs_in[token]
                        eo_prev = eo_tiles.pop(cb - 1)
                        for ct in range(CBLK // 128):
                            kprev = (CBLK // 128) * (cb - 1) + ct
                            nc.gpsimd.indirect_dma_start(
                                out=scat_view, in_=eo_prev[:, ct, :], in_offset=None,
                                out_offset=bass.IndirectOffsetOnAxis(ap=tokci[:, kprev : kprev + 1], axis=0),
                                bounds_check=T - 1, oob_is_err=False,
                            )

                    hT = pf.tile([128, F // 128, CBLK], BF16, tag="hT")
                    for ft in range(F // 128):
                        ps1 = psm.tile([128, CBLK], F32, space="PSUM", tag="ps1")
                        for kd in range(D // 128):
                            nc.tensor.matmul(
                                ps1[:],
                                lhsT=wgt_sb[:, kd, 128 * ft : 128 * ft + 128],
                                rhs=dispT[:, kd, :],
                                start=(kd == 0), stop=(kd == D // 128 - 1),
                            )
                        nc.scalar.activation(hT[:, ft, :], ps1[:], AF.Gelu)
                    if cb == 0:
                        # gate merge + diagonal: created here so the merge mms
                        # sit behind mm1(b0) in the in-order PE queue (ready
                        # well before PE reaches them - no head-of-line stall),
                        # while gsl is still written before its first reader
                        gscr = pf.tile([128, 128], BF16)
                        for half, gdst in ((0, gdst_lo), (1, gdst_hi)):
                            for ch in range(2):
                                gm_ps = pss.tile([128, 512], F32, space="PSUM", tag="ps_small")
                                nc.tensor.matmul(gm_ps[:], lhsT=ones_bf[:], rhs=gdst[:, 512 * ch : 512 * (ch + 1)], start=True, stop=True)
                                for kk in range(4):
                                    k = (2 * half + ch) * 4 + kk
                                    nc.vector.scalar_tensor_tensor(
                                        out=gscr[:], in0=gm_ps[:, 128 * kk : 128 * (kk + 1)], scalar=0.0,
                                        in1=identp[:], op0=OP.add, op1=OP.mult,
                                        accum_out=gsl[:, k : k + 1],
                                    )
                    # mm2 with swapped operands: eo[c, d] = g * (hT.T @ w_down)
                    eo_sb = pfd.tile([128, CBLK // 128, D], BF16, tag="eo_sb")
                    eo_tiles[cb] = eo_sb
                    for ct in range(CBLK // 128):
                        kcur = (CBLK // 128) * cb + ct
                        for dc in range(D // 512):
                            ps2 = psm.tile([128, 512], F32, space="PSUM", tag="ps2")
                            for ft in range(F // 128):
                                nc.tensor.matmul(
                                    ps2[:],
                                    lhsT=hT[:, ft, 128 * ct : 128 * ct + 128],
                                    rhs=wdn_sb[:, ft, 512 * dc : 512 * dc + 512],
                                    start=(ft == 0), stop=(ft == F // 128 - 1),
                                )
                            nc.vector.tensor_scalar_mul(
                                eo_sb[:, ct, 512 * dc : 512 * dc + 512], ps2[:],
                                gsl[:, kcur : kcur + 1],
                            )
                        if cb == NCB - 1:
                            # last block: scatter each ct as soon as it is
                            # gated, so the RS waits only on the final ct
                            nc.gpsimd.indirect_dma_start(
                                out=scat_view, in_=eo_sb[:, ct, :], in_offset=None,
                                out_offset=bass.IndirectOffsetOnAxis(ap=tokci[:, kcur : kcur + 1], axis=0),
                                bounds_check=T - 1, oob_is_err=False,
                            )
                eo_tiles.pop(NCB - 1)

                # sum the two expert contributions per token; shard m -> core m
                nc.gpsimd.collective_compute(
                    "ReduceScatter", OP.add,
                    replica_groups=[list(range(NC))],
                    ins=[rs_in[:].opt()], outs=[rs_out[:].opt()],
                )
                nc.sync.dma_start(y_d[:], rs_out[:])

    nc.compile()
    return nc


_PROGRAM = None


def _get_program():
    global _PROGRAM
    if _PROGRAM is None:
        _PROGRAM = _build_program()
    return _PROGRAM


def host_constants():
    p = np.arange(128)
    return {
        "ident": np.eye(128, dtype=np.float32),
        "slmat": (np.arange(128)[None, :] > p[:, None]).astype(np.float32),
        "tidx": (64 * p[:, None] + np.arange(64)[None, :]).astype(np.float32),
        "eidx": np.tile(np.arange(E, dtype=np.float32), (128, 1)),
    }


def _make_in_maps(x, wg, w_gate, w_down):
    x = np.asarray(x, np.float32)
    wg_np = np.asarray(wg, np.float32)
    w_gate_np = np.asarray(w_gate, np.float32)
    w_down_np = np.asarray(w_down, np.float32)

    tokens = x.reshape(T, D)
    xb = np.zeros((T + 1, D), ml_dtypes.bfloat16)
    xb[:T] = tokens.astype(ml_dtypes.bfloat16)

    # shard m holds tokens [SH*m, SH*(m+1)); its xT columns are permuted so that
    # matmul tile position j = 128*tt + p corresponds to local token 8*p + tt,
    # making the routing payload DMA contiguous.
    j = np.arange(SH)
    perm = 8 * (j % 128) + j // 128  # local token index at column position j
    consts = host_constants()

    in_maps = []
    for m in range(NC):
        shard = tokens[SH * m : SH * (m + 1)]
        xT_sh = np.ascontiguousarray(shard[perm].T)
        in_maps.append({
            "xT_sh": xT_sh,
            "xb": xb,
            "wg": wg_np,
            "wgt": np.ascontiguousarray(w_gate_np[m].astype(ml_dtypes.bfloat16)),
            "wdn": np.ascontiguousarray(w_down_np[m].astype(ml_dtypes.bfloat16)),
            "cid": np.full((128, 1), float(m), np.float32),
            **consts,
        })
    return in_maps


def kernel(x, wg, w_gate, w_down, _trace=False):
    global LAST_RESULT
    x = np.asarray(x, np.float32)
    in_maps = _make_in_maps(x, wg, w_gate, w_down)
    nc = _get_program()
    res = run_bass_kernel_spmd(nc, in_maps, core_ids=list(range(NC)), trace=_trace)
    LAST_RESULT = res
    out = np.concatenate([res.results[m]["y"] for m in range(NC)], axis=0)
    return out.reshape(B, S, D).astype(x.dtype)


def bench(x, wg, w_gate, w_down, iters=6):
    """Measure per-execution wall time with device-resident inputs.

    Returns (output, per_call_seconds_list).
    """
    import time
    import jax
    from jax.sharding import Mesh, PartitionSpec, NamedSharding
    from jax.experimental.shard_map import shard_map
    import concourse.mybir as _mybir
    from concourse.bass2jax import _bass_exec_p, install_neuronx_cc_hook, partition_id_tensor

    install_neuronx_cc_hook()
    nc = _get_program()

    x = np.asarray(x, np.float32)
    in_maps = _make_in_maps(x, wg, w_gate, w_down)

    in_names, out_names, out_avals, zero_outs = [], [], [], []
    for alloc in nc.m.functions[0].allocations:
        if not isinstance(alloc, _mybir.MemoryLocationSet):
            continue
        name = alloc.memorylocations[0].name
        if alloc.kind == "ExternalInput":
            if nc.partition_id_tensor is None or name != nc.partition_id_tensor.name:
                in_names.append(name)
        elif alloc.kind == "ExternalOutput":
            shape = tuple(alloc.tensor_shape)
            dtype = _mybir.dt.np(alloc.dtype)
            out_names.append(name)
            out_avals.append(jax.core.ShapedArray(shape, dtype))
            zero_outs.append(np.zeros(shape, dtype))
    n_params = len(in_names)
    all_in_names = in_names + out_names
    if nc.partition_id_tensor is not None:
        all_in_names = all_in_names + [nc.partition_id_tensor.name]

    def _body(*args):
        operands = list(args)
        if nc.partition_id_tensor is not None:
            operands.append(partition_id_tensor())
        outs = _bass_exec_p.bind(
            *operands,
            out_avals=tuple(out_avals),
            in_names=tuple(all_in_names),
            out_names=tuple(out_names),
            lowering_input_output_aliases=(),
            sim_require_finite=True,
            sim_require_nnan=True,
            nc=nc,
        )
        return tuple(outs)

    devices = jax.devices()[:NC]
    mesh = Mesh(np.asarray(devices), ("core",))
    nsh = NamedSharding(mesh, PartitionSpec("core"))
    n_outs = len(out_avals)
    donate = tuple(range(n_params, n_params + n_outs))
    sharded = jax.jit(
        shard_map(_body, mesh=mesh, in_specs=(PartitionSpec("core"),) * (n_params + n_outs),
                  out_specs=(PartitionSpec("core"),) * n_outs, check_rep=False),
        donate_argnums=donate, keep_unused=True,
    )

    concat_in = [
        jax.device_put(np.concatenate([np.asarray(in_maps[c][nm]) for c in range(NC)], axis=0), nsh)
        for nm in in_names
    ]
    zero_sets = [
        [jax.device_put(np.zeros((NC * z.shape[0], *z.shape[1:]), z.dtype), nsh) for z in zero_outs]
        for _ in range(iters + 1)
    ]

    out = sharded(*concat_in, *zero_sets[0])  # warmup + compile
    jax.block_until_ready(out)
    times = []
    for it in range(iters):
        t0 = time.perf_counter()
        out = sharded(*concat_in, *zero_sets[it + 1])
        jax.block_until_ready(out)
        times.append(time.perf_counter() - t0)

    outs = {
        nm: np.asarray(out[i]).reshape(NC, *out_avals[i].shape) for i, nm in enumerate(out_names)
    }
    y = np.concatenate([outs["y"][m] for m in range(NC)], axis=0).reshape(B, S, D).astype(x.dtype)
    return y, times
